# revision 10
# baseline (speedup 1.0000x reference)
"""Block-diagonal attention kernel for Trainium2 (8 NeuronCores).

Problem: q,k,v [4, 16, 4128, 64] f32. For each (b,h): attention is computed
independently within consecutive 64-row blocks (64 full blocks) plus one
final 32-row block (4128 = 64*64 + 32).

Sharding: B*H = 64 (b,h) pairs -> 8 heads per core (pure data parallel).

v2 design (vs the fp32/PE-transpose baseline at ~180-195 us):
  - Host pre-permutes q,k into [head-pair, 128, 4096] where partitions are
    (head0 d 0:64 | head1 d 64:128) and the free dim is token rows. Q^T/K^T
    then DMA straight from HBM with ONE contiguous 16KB descriptor per
    partition (vs 32x 256B runs), and the PE transposes + transpose PSUM
    bank + PSUM->SBUF copy of the baseline disappear entirely.
  - v is host-permuted to [head, p, chunk, 65] with a ones column baked in
    (denominator trick) and loaded via gpsimd cast-DMA straight to bf16.
  - q,k are cast fp32->bf16 on DVE (one tensor_copy per 2MB pair tile).
    All matmuls run in bf16: 1 PE cycle/row instead of 4 (fp32).
  - exp writes bf16 pt tiles; their off-diagonal (cross-block garbage)
    quadrants are zeroed once at startup and never written again.
  - Output is written bf16 to HBM in [head, p, chunk, d] layout (contiguous
    4KB/partition descriptors), upcast + unpermuted on host.
Expected: DMA-bound at ~24.7 MB in + 4.3 MB out per core.
"""
import sys

sys.path.insert(0, "/opt/trn_rl_repo")

import numpy as np
from contextlib import ExitStack

import concourse.tile as tile
from concourse import bacc, mybir
from concourse.bass_utils import run_bass_kernel_spmd
from concourse.masks import make_identity

F32 = mybir.dt.float32
BF16 = mybir.dt.bfloat16
AF = mybir.ActivationFunctionType

B, H, N, D = 4, 16, 4128, 64
BH_PER_CORE = 8          # 64 (b,h) pairs / 8 cores
N_PAIR = 4               # head pairs per core
NMAIN = 4096             # rows covered by full 64-blocks, per (b,h)
NREM = 32                # remainder block rows
SC = 512                 # superchunk rows (4 chunks of 128)
N_SC = NMAIN // SC       # 8 superchunks per (b,h)
SCALE = 1.0 / 8.0        # 1/sqrt(D)

PSUM_BUFS = {"ss": 2, "o": 3}
PT_BUFS = 3
REMAINDER_AFTER_BH = 1   # hide the remainder's serial chain under main DMA
BIG_BUFS = 3
SB_BUFS = 3


def _superchunk(nc, pools, qt16, kt16, p0, vsb, outb, s):
    sb, ps = pools

    # S^T per chunk; chunk c -> cols 512*(c%2) + 128*(c//2) of 2-bank tile.
    # All 4 matmuls share row group p0 -> serialized on PE -> bank-safe.
    ss = ps.tile([128, 1024], F32, tag="ss", bufs=PSUM_BUFS["ss"])
    for c in range(4):
        col = 512 * (c % 2) + 128 * (c // 2)
        n0 = SC * s + 128 * c
        nc.tensor.matmul(ss[:, col:col + 128],
                         kt16[p0:p0 + 64, n0:n0 + 128],
                         qt16[p0:p0 + 64, n0:n0 + 128],
                         tile_position=(p0, 0))

    # exp of diagonal quadrants -> pt (col groups ordered [c0|c2|c1|c3]);
    # off-diagonal quadrants stay at their startup-zeroed value.
    pt = sb.tile([128, 512], BF16, tag="pt", bufs=PT_BUFS)
    ptq = pt.rearrange("p (b g t d) -> p b g t d", b=2, g=2, t=2)
    ssq = ss.rearrange("p (b u g t d) -> p b u g t d", b=2, u=2, g=2, t=2)[:, :, 0]
    nc.scalar.activation(ptq[0:64, :, :, 0, :], ssq[0:64, :, :, 0, :], AF.Exp,
                         scale=SCALE)
    nc.scalar.activation(ptq[64:128, :, :, 1, :], ssq[64:128, :, :, 1, :], AF.Exp,
                         scale=SCALE)

    # PV per chunk: o[:, c, :] = P_c^T.T @ [V_c | 1]
    o = ps.tile([128, 4, 65], F32, tag="o", bufs=PSUM_BUFS["o"])
    for c in range(4):
        j = 2 * (c % 2) + c // 2
        nc.tensor.matmul(o[:, c, :], pt[:, 128 * j:128 * j + 128],
                         vsb[:, 4 * s + c, :])

    # normalize into the per-head output tile: one DVE op per superchunk
    # (out = o * (1/sum) with the per-chunk scale broadcast along d)
    r = sb.tile([128, 4], F32, tag="r")
    nc.vector.reciprocal(r[:], o[:, :, 64])
    nc.vector.tensor_tensor(
        outb[:, 4 * s:4 * s + 4, :], o[:, :, 0:64],
        r[:, :, None].to_broadcast((128, 4, 64)), mybir.AluOpType.mult)


def _remainder(nc, pools, ident, qr, kr, vr, orr):
    """All 8 bh remainder blocks ([32, 64] each) in one pass on partitions
    0:32, blocks stacked along the free dim. All matmuls share row group ->
    serialized -> single-bank PSUM writes are safe."""
    sb, ps = pools

    rq = sb.tile([32, 8, 64], F32, tag="rq")
    rk = sb.tile([32, 8, 64], F32, tag="rk")
    rv = sb.tile([32, 8, 65], BF16, tag="rv")
    nc.sync.dma_start(out=rq[:], in_=qr[:])
    nc.scalar.dma_start(out=rk[:], in_=kr[:])
    nc.gpsimd.dma_start(out=rv[:], in_=vr[:])  # casts f32 -> bf16 in flight

    # transposes: rt[64, 0:256]=Q^T (8x[64,32]), [64, 256:512]=K^T
    rt = ps.tile([64, 512], F32, tag="ss", bufs=PSUM_BUFS["ss"])
    for j in range(8):
        nc.tensor.transpose(rt[:, 32 * j:32 * j + 32], rq[:, j, :],
                            ident[0:32, 0:32])
        nc.tensor.transpose(rt[:, 256 + 32 * j:256 + 32 * j + 32], rk[:, j, :],
                            ident[0:32, 0:32])
    rts = sb.tile([64, 512], BF16, tag="rts")
    nc.vector.tensor_copy(rts[:], rt[:])

    # S^T per block: [32, 32] at partitions 0:32, serialized, one bank
    rss = ps.tile([32, 8, 32], F32, tag="o", bufs=PSUM_BUFS["o"])
    for j in range(8):
        nc.tensor.matmul(rss[:, j, :], rts[:, 256 + 32 * j:256 + 32 * j + 32],
                         rts[:, 32 * j:32 * j + 32])

    rpt = sb.tile([32, 8, 32], BF16, tag="rpt")
    nc.scalar.activation(rpt[:], rss[:], AF.Exp, scale=SCALE)

    # PV per block: [32, 65] at cols 128j of a 2-bank tile (no crossing)
    ro = ps.tile([32, 8, 128], F32, tag="ss", bufs=PSUM_BUFS["ss"])
    for j in range(8):
        nc.tensor.matmul(ro[:, j, 0:65], rpt[:, j, :], rv[:, j, :])

    rr = sb.tile([32, 8], F32, tag="rr")
    nc.vector.reciprocal(rr[:], ro[:, :, 64])
    routs = sb.tile([32, 8, 64], BF16, tag="routs")
    nc.vector.tensor_tensor(
        routs[:], ro[:, :, 0:64],
        rr[:, :, None].to_broadcast((32, 8, 64)), mybir.AluOpType.mult)

    nc.sync.dma_start(out=orr[:], in_=routs[:])


def build_nc(repeat=1, mode="full"):
    """mode='full': the real kernel. mode='dma': loads + stores only
    (outb/routs memset once) — measures the achievable DMA floor."""
    nc = bacc.Bacc("TRN2", target_bir_lowering=False, debug=False, num_devices=8)
    qt = nc.dram_tensor("qt", [N_PAIR, 128, NMAIN], F32, kind="ExternalInput").ap()
    kt = nc.dram_tensor("kt", [N_PAIR, 128, NMAIN], F32, kind="ExternalInput").ap()
    vp = nc.dram_tensor("vp", [BH_PER_CORE, 128, 32, 65], F32,
                        kind="ExternalInput").ap()
    qr = nc.dram_tensor("qr", [NREM, 8, 64], F32, kind="ExternalInput").ap()
    kr = nc.dram_tensor("kr", [NREM, 8, 64], F32, kind="ExternalInput").ap()
    vr = nc.dram_tensor("vr", [NREM, 8, 65], F32, kind="ExternalInput").ap()
    om = nc.dram_tensor("om", [BH_PER_CORE, 128, 32, 64], BF16,
                        kind="ExternalOutput").ap()
    orr = nc.dram_tensor("orr", [NREM, 8, 64], BF16, kind="ExternalOutput").ap()

    with tile.TileContext(nc) as tc, ExitStack() as ctx:
        singles = ctx.enter_context(tc.tile_pool(name="singles", bufs=1))
        big = ctx.enter_context(tc.tile_pool(name="big", bufs=BIG_BUFS))
        sb = ctx.enter_context(tc.tile_pool(name="sb", bufs=SB_BUFS))
        ps = ctx.enter_context(tc.tile_pool(name="ps", bufs=2, space="PSUM"))
        pools = (sb, ps)

        ident = singles.tile([128, 128], F32)
        make_identity(nc, ident[:])

        # pre-zero every pt slot; the loop's pt tiles reuse these slots
        # round-robin and exp never writes the off-diagonal quadrants
        for _ in range(PT_BUFS):
            pt0 = sb.tile([128, 512], BF16, tag="pt", bufs=PT_BUFS)
            nc.gpsimd.memset(pt0[:], 0.0)

        for _ in range(repeat):
            for pair in range(N_PAIR):
                # SWDGE cast-DMA: f32 HBM reads land as bf16 in SBUF, no
                # on-chip cast pass. All input loads ride the SWDGE queue;
                # output stores ride the two HWDGE rings.
                qt16 = big.tile([128, NMAIN], BF16, tag="qt16")
                kt16 = big.tile([128, NMAIN], BF16, tag="kt16")
                nc.gpsimd.dma_start(out=qt16[:], in_=qt[pair])
                nc.gpsimd.dma_start(out=kt16[:], in_=kt[pair])
                for hh in range(2):
                    head = 2 * pair + hh
                    p0 = 64 * hh
                    vsb = big.tile([128, 32, 65], BF16, tag="vsb")
                    nc.gpsimd.dma_start(out=vsb[:], in_=vp[head])  # casts
                    outb = big.tile([128, 32, 64], BF16, tag="outb")
                    if mode == "full":
                        for s in range(N_SC):
                            _superchunk(nc, pools, qt16, kt16, p0, vsb, outb, s)
                    else:
                        nc.gpsimd.memset(outb[:], 0.0)
                    out_eng = nc.sync if head % 2 == 0 else nc.scalar
                    out_eng.dma_start(out=om[head], in_=outb[:])
                    if REMAINDER_AFTER_BH == head and mode == "full":
                        _remainder(nc, pools, ident, qr, kr, vr, orr)
            if mode != "full":
                rq = sb.tile([32, 8, 64], F32, tag="rq")
                rk = sb.tile([32, 8, 64], F32, tag="rk")
                rv = sb.tile([32, 8, 65], BF16, tag="rv")
                nc.sync.dma_start(out=rq[:], in_=qr[:])
                nc.scalar.dma_start(out=rk[:], in_=kr[:])
                nc.gpsimd.dma_start(out=rv[:], in_=vr[:])
                routs = sb.tile([32, 8, 64], BF16, tag="routs")
                nc.gpsimd.memset(routs[:], 0.0)
                nc.sync.dma_start(out=orr[:], in_=routs[:])
            elif REMAINDER_AFTER_BH is None:
                _remainder(nc, pools, ident, qr, kr, vr, orr)

    nc.compile()
    return nc


def pack_full_inputs(q, k, v):
    """Host-side permute of full [B,H,N,D] inputs into the device HBM
    layouts, as full arrays whose axis 0 concatenates the 8 cores."""
    q64 = np.asarray(q, dtype=np.float32).reshape(B * H, N, D)
    k64 = np.asarray(k, dtype=np.float32).reshape(B * H, N, D)
    v64 = np.asarray(v, dtype=np.float32).reshape(B * H, N, D)

    def t_main(x):  # [64, 4096, 64] -> [32 pairs, 128, 4096]
        return np.ascontiguousarray(
            x[:, :NMAIN, :].transpose(0, 2, 1)).reshape(32, 128, NMAIN)

    def rem(x, pad):  # [64, 32, 64] -> [256, 8, 64(+1)]
        r = x[:, NMAIN:, :].reshape(8, 8, NREM, D).transpose(0, 2, 1, 3)
        if pad:
            rp = np.empty((8, NREM, 8, D + 1), np.float32)
            rp[..., :D] = r
            rp[..., D] = 1.0
            r = rp
        return np.ascontiguousarray(r).reshape(8 * NREM, 8, D + (1 if pad else 0))

    vm = v64[:, :NMAIN, :].reshape(64, 32, 128, D).transpose(0, 2, 1, 3)
    vpf = np.empty((64, 128, 32, D + 1), np.float32)
    vpf[..., :D] = vm
    vpf[..., D] = 1.0

    return {
        "qt": t_main(q64), "kt": t_main(k64), "vp": vpf,
        "qr": rem(q64, False), "kr": rem(k64, False), "vr": rem(v64, True),
    }


def unpack_full_outputs(om_full, orr_full):
    """om [64,128,32,64] bf16, orr [256,8,64] bf16 -> [B,H,N,D] f32."""
    main = om_full.astype(np.float32).transpose(0, 2, 1, 3).reshape(64, NMAIN, D)
    rem = orr_full.astype(np.float32).reshape(8, NREM, 8, D).transpose(
        0, 2, 1, 3).reshape(64, NREM, D)
    return np.concatenate([main, rem], axis=1).reshape(B, H, N, D)


_CACHE = {}


def kernel(q, k, v):
    assert q.shape == (B, H, N, D), q.shape
    if "nc" not in _CACHE:
        _CACHE["nc"] = build_nc()
    nc = _CACHE["nc"]

    full = pack_full_inputs(q, k, v)
    in_maps = []
    for i in range(8):
        in_maps.append({
            "qt": full["qt"][N_PAIR * i:N_PAIR * (i + 1)],
            "kt": full["kt"][N_PAIR * i:N_PAIR * (i + 1)],
            "vp": full["vp"][BH_PER_CORE * i:BH_PER_CORE * (i + 1)],
            "qr": full["qr"][NREM * i:NREM * (i + 1)],
            "kr": full["kr"][NREM * i:NREM * (i + 1)],
            "vr": full["vr"][NREM * i:NREM * (i + 1)],
        })

    # Retries: rapid repeated executions occasionally wedge a core with a
    # transient NRT_EXEC_UNIT_UNRECOVERABLE; a fresh attempt recovers.
    import time
    res = None
    for attempt in range(3):
        try:
            res = run_bass_kernel_spmd(nc, in_maps, core_ids=list(range(8)))
            break
        except Exception:
            if attempt == 2:
                raise
            time.sleep(3.0)

    om_full = np.concatenate([np.asarray(res.results[i]["om"]) for i in range(8)])
    orr_full = np.concatenate([np.asarray(res.results[i]["orr"]) for i in range(8)])
    return unpack_full_outputs(om_full, orr_full)


# revision 11
# speedup vs baseline: 3.3783x; 3.3783x over previous
"""Block-diagonal attention kernel for Trainium2 (8 NeuronCores).

Problem: q,k,v [4, 16, 4128, 64] f32. For each (b,h): attention is computed
independently within consecutive 64-row blocks (64 full blocks) plus one
final 32-row block (4128 = 64*64 + 32).

Sharding: B*H = 64 (b,h) pairs -> 8 heads per core (pure data parallel).

v2 design (vs the fp32/PE-transpose baseline at ~180-195 us):
  - Host pre-permutes q,k into [head-pair, 128, 4096] where partitions are
    (head0 d 0:64 | head1 d 64:128) and the free dim is token rows. Q^T/K^T
    then DMA straight from HBM with ONE contiguous 16KB descriptor per
    partition (vs 32x 256B runs), and the PE transposes + transpose PSUM
    bank + PSUM->SBUF copy of the baseline disappear entirely.
  - v is host-permuted to [head, p, chunk, 65] with a ones column baked in
    (denominator trick) and loaded via gpsimd cast-DMA straight to bf16.
  - q,k are cast fp32->bf16 on DVE (one tensor_copy per 2MB pair tile).
    All matmuls run in bf16: 1 PE cycle/row instead of 4 (fp32).
  - exp writes bf16 pt tiles; their off-diagonal (cross-block garbage)
    quadrants are zeroed once at startup and never written again.
  - Output is written bf16 to HBM in [head, p, chunk, d] layout (contiguous
    4KB/partition descriptors), upcast + unpermuted on host.
Expected: DMA-bound at ~24.7 MB in + 4.3 MB out per core.
"""
import sys

sys.path.insert(0, "/opt/trn_rl_repo")

import numpy as np
from contextlib import ExitStack

import concourse.tile as tile
from concourse import bacc, mybir
from concourse.bass_utils import run_bass_kernel_spmd
from concourse.masks import make_identity

F32 = mybir.dt.float32
BF16 = mybir.dt.bfloat16
AF = mybir.ActivationFunctionType

B, H, N, D = 4, 16, 4128, 64
BH_PER_CORE = 8          # 64 (b,h) pairs / 8 cores
N_PAIR = 4               # head pairs per core
NMAIN = 4096             # rows covered by full 64-blocks, per (b,h)
NREM = 32                # remainder block rows
SC = 512                 # superchunk rows (4 chunks of 128)
N_SC = NMAIN // SC       # 8 superchunks per (b,h)
SCALE = 1.0 / 8.0        # 1/sqrt(D)

PSUM_BUFS = {"ss": 2, "o": 3}
PT_BUFS = 3
REMAINDER_AFTER_BH = 1   # hide the remainder's serial chain under main DMA
BIG_BUFS = 3
SB_BUFS = 3


def _superchunk(nc, pools, qt16, kt16, p0, vsb, outb, s):
    sb, ps = pools

    # S^T per chunk; chunk c -> cols 512*(c%2) + 128*(c//2) of 2-bank tile.
    # All 4 matmuls share row group p0 -> serialized on PE -> bank-safe.
    ss = ps.tile([128, 1024], F32, tag="ss", bufs=PSUM_BUFS["ss"])
    for c in range(4):
        col = 512 * (c % 2) + 128 * (c // 2)
        n0 = SC * s + 128 * c
        nc.tensor.matmul(ss[:, col:col + 128],
                         kt16[p0:p0 + 64, n0:n0 + 128],
                         qt16[p0:p0 + 64, n0:n0 + 128],
                         tile_position=(p0, 0))

    # exp of diagonal quadrants -> pt (col groups ordered [c0|c2|c1|c3]);
    # off-diagonal quadrants stay at their startup-zeroed value.
    pt = sb.tile([128, 512], BF16, tag="pt", bufs=PT_BUFS)
    ptq = pt.rearrange("p (b g t d) -> p b g t d", b=2, g=2, t=2)
    ssq = ss.rearrange("p (b u g t d) -> p b u g t d", b=2, u=2, g=2, t=2)[:, :, 0]
    nc.scalar.activation(ptq[0:64, :, :, 0, :], ssq[0:64, :, :, 0, :], AF.Exp,
                         scale=SCALE)
    nc.scalar.activation(ptq[64:128, :, :, 1, :], ssq[64:128, :, :, 1, :], AF.Exp,
                         scale=SCALE)

    # PV per chunk: o[:, c, :] = P_c^T.T @ [V_c | 1]
    o = ps.tile([128, 4, 65], F32, tag="o", bufs=PSUM_BUFS["o"])
    for c in range(4):
        j = 2 * (c % 2) + c // 2
        nc.tensor.matmul(o[:, c, :], pt[:, 128 * j:128 * j + 128],
                         vsb[:, 4 * s + c, :])

    # normalize into the per-head output tile: one DVE op per superchunk
    # (out = o * (1/sum) with the per-chunk scale broadcast along d)
    r = sb.tile([128, 4], F32, tag="r")
    nc.vector.reciprocal(r[:], o[:, :, 64])
    nc.vector.tensor_tensor(
        outb[:, 4 * s:4 * s + 4, :], o[:, :, 0:64],
        r[:, :, None].to_broadcast((128, 4, 64)), mybir.AluOpType.mult)


def _remainder(nc, pools, ident, qr, kr, vr, orr):
    """All 8 bh remainder blocks ([32, 64] each) in one pass on partitions
    0:32, blocks stacked along the free dim. All matmuls share row group ->
    serialized -> single-bank PSUM writes are safe."""
    sb, ps = pools

    rq = sb.tile([32, 8, 64], F32, tag="rq")
    rk = sb.tile([32, 8, 64], F32, tag="rk")
    rv = sb.tile([32, 8, 65], BF16, tag="rv")
    nc.sync.dma_start(out=rq[:], in_=qr[:])
    nc.scalar.dma_start(out=rk[:], in_=kr[:])
    nc.gpsimd.dma_start(out=rv[:], in_=vr[:])  # casts f32 -> bf16 in flight

    # transposes: rt[64, 0:256]=Q^T (8x[64,32]), [64, 256:512]=K^T
    rt = ps.tile([64, 512], F32, tag="ss", bufs=PSUM_BUFS["ss"])
    for j in range(8):
        nc.tensor.transpose(rt[:, 32 * j:32 * j + 32], rq[:, j, :],
                            ident[0:32, 0:32])
        nc.tensor.transpose(rt[:, 256 + 32 * j:256 + 32 * j + 32], rk[:, j, :],
                            ident[0:32, 0:32])
    rts = sb.tile([64, 512], BF16, tag="rts")
    nc.vector.tensor_copy(rts[:], rt[:])

    # S^T per block: [32, 32] at partitions 0:32, serialized, one bank
    rss = ps.tile([32, 8, 32], F32, tag="o", bufs=PSUM_BUFS["o"])
    for j in range(8):
        nc.tensor.matmul(rss[:, j, :], rts[:, 256 + 32 * j:256 + 32 * j + 32],
                         rts[:, 32 * j:32 * j + 32])

    rpt = sb.tile([32, 8, 32], BF16, tag="rpt")
    nc.scalar.activation(rpt[:], rss[:], AF.Exp, scale=SCALE)

    # PV per block: [32, 65] at cols 128j of a 2-bank tile (no crossing)
    ro = ps.tile([32, 8, 128], F32, tag="ss", bufs=PSUM_BUFS["ss"])
    for j in range(8):
        nc.tensor.matmul(ro[:, j, 0:65], rpt[:, j, :], rv[:, j, :])

    rr = sb.tile([32, 8], F32, tag="rr")
    nc.vector.reciprocal(rr[:], ro[:, :, 64])
    routs = sb.tile([32, 8, 64], BF16, tag="routs")
    nc.vector.tensor_tensor(
        routs[:], ro[:, :, 0:64],
        rr[:, :, None].to_broadcast((32, 8, 64)), mybir.AluOpType.mult)

    nc.sync.dma_start(out=orr[:], in_=routs[:])


def build_nc(repeat=1, mode="full"):
    """mode='full': the real kernel. mode='dma': loads + stores only
    (outb/routs memset once) — measures the achievable DMA floor."""
    nc = bacc.Bacc("TRN2", target_bir_lowering=False, debug=False, num_devices=8)
    qt = nc.dram_tensor("qt", [N_PAIR, 128, NMAIN], F32, kind="ExternalInput").ap()
    kt = nc.dram_tensor("kt", [N_PAIR, 128, NMAIN], F32, kind="ExternalInput").ap()
    vp = nc.dram_tensor("vp", [BH_PER_CORE, 128, 32, 65], F32,
                        kind="ExternalInput").ap()
    qr = nc.dram_tensor("qr", [NREM, 8, 64], F32, kind="ExternalInput").ap()
    kr = nc.dram_tensor("kr", [NREM, 8, 64], F32, kind="ExternalInput").ap()
    vr = nc.dram_tensor("vr", [NREM, 8, 65], F32, kind="ExternalInput").ap()
    om = nc.dram_tensor("om", [BH_PER_CORE, 128, 32, 64], BF16,
                        kind="ExternalOutput").ap()
    orr = nc.dram_tensor("orr", [NREM, 8, 64], BF16, kind="ExternalOutput").ap()

    with tile.TileContext(nc) as tc, ExitStack() as ctx:
        singles = ctx.enter_context(tc.tile_pool(name="singles", bufs=1))
        big = ctx.enter_context(tc.tile_pool(name="big", bufs=BIG_BUFS))
        sb = ctx.enter_context(tc.tile_pool(name="sb", bufs=SB_BUFS))
        ps = ctx.enter_context(tc.tile_pool(name="ps", bufs=2, space="PSUM"))
        pools = (sb, ps)

        ident = singles.tile([128, 128], F32)
        make_identity(nc, ident[:])

        # pre-zero every pt slot; the loop's pt tiles reuse these slots
        # round-robin and exp never writes the off-diagonal quadrants
        for _ in range(PT_BUFS):
            pt0 = sb.tile([128, 512], BF16, tag="pt", bufs=PT_BUFS)
            nc.gpsimd.memset(pt0[:], 0.0)

        for _ in range(repeat):
            for pair in range(N_PAIR):
                # SWDGE cast-DMA: f32 HBM reads land as bf16 in SBUF, no
                # on-chip cast pass. All input loads ride the SWDGE queue;
                # output stores ride the two HWDGE rings.
                qt16 = big.tile([128, NMAIN], BF16, tag="qt16")
                kt16 = big.tile([128, NMAIN], BF16, tag="kt16")
                nc.gpsimd.dma_start(out=qt16[:], in_=qt[pair])
                nc.gpsimd.dma_start(out=kt16[:], in_=kt[pair])
                for hh in range(2):
                    head = 2 * pair + hh
                    p0 = 64 * hh
                    vsb = big.tile([128, 32, 65], BF16, tag="vsb")
                    nc.gpsimd.dma_start(out=vsb[:], in_=vp[head])  # casts
                    outb = big.tile([128, 32, 64], BF16, tag="outb")
                    if mode == "full":
                        for s in range(N_SC):
                            _superchunk(nc, pools, qt16, kt16, p0, vsb, outb, s)
                    else:
                        nc.gpsimd.memset(outb[:], 0.0)
                    out_eng = nc.sync if head % 2 == 0 else nc.scalar
                    out_eng.dma_start(out=om[head], in_=outb[:])
                    if REMAINDER_AFTER_BH == head and mode == "full":
                        _remainder(nc, pools, ident, qr, kr, vr, orr)
            if mode != "full":
                rq = sb.tile([32, 8, 64], F32, tag="rq")
                rk = sb.tile([32, 8, 64], F32, tag="rk")
                rv = sb.tile([32, 8, 65], BF16, tag="rv")
                nc.sync.dma_start(out=rq[:], in_=qr[:])
                nc.scalar.dma_start(out=rk[:], in_=kr[:])
                nc.gpsimd.dma_start(out=rv[:], in_=vr[:])
                routs = sb.tile([32, 8, 64], BF16, tag="routs")
                nc.gpsimd.memset(routs[:], 0.0)
                nc.sync.dma_start(out=orr[:], in_=routs[:])
            elif REMAINDER_AFTER_BH is None:
                _remainder(nc, pools, ident, qr, kr, vr, orr)

    nc.compile()
    return nc


def pack_full_inputs(q, k, v):
    """Host-side permute of full [B,H,N,D] inputs into the device HBM
    layouts, as full arrays whose axis 0 concatenates the 8 cores."""
    q64 = np.asarray(q, dtype=np.float32).reshape(B * H, N, D)
    k64 = np.asarray(k, dtype=np.float32).reshape(B * H, N, D)
    v64 = np.asarray(v, dtype=np.float32).reshape(B * H, N, D)

    def t_main(x):  # [64, 4096, 64] -> [32 pairs, 128, 4096]
        return np.ascontiguousarray(
            x[:, :NMAIN, :].transpose(0, 2, 1)).reshape(32, 128, NMAIN)

    def rem(x, pad):  # [64, 32, 64] -> [256, 8, 64(+1)]
        r = x[:, NMAIN:, :].reshape(8, 8, NREM, D).transpose(0, 2, 1, 3)
        if pad:
            rp = np.empty((8, NREM, 8, D + 1), np.float32)
            rp[..., :D] = r
            rp[..., D] = 1.0
            r = rp
        return np.ascontiguousarray(r).reshape(8 * NREM, 8, D + (1 if pad else 0))

    vm = v64[:, :NMAIN, :].reshape(64, 32, 128, D).transpose(0, 2, 1, 3)
    vpf = np.empty((64, 128, 32, D + 1), np.float32)
    vpf[..., :D] = vm
    vpf[..., D] = 1.0

    return {
        "qt": t_main(q64), "kt": t_main(k64), "vp": vpf,
        "qr": rem(q64, False), "kr": rem(k64, False), "vr": rem(v64, True),
    }


def unpack_full_outputs(om_full, orr_full):
    """om [64,128,32,64] bf16, orr [256,8,64] bf16 -> [B,H,N,D] f32."""
    main = om_full.astype(np.float32).transpose(0, 2, 1, 3).reshape(64, NMAIN, D)
    rem = orr_full.astype(np.float32).reshape(8, NREM, 8, D).transpose(
        0, 2, 1, 3).reshape(64, NREM, D)
    return np.concatenate([main, rem], axis=1).reshape(B, H, N, D)


_CACHE = {}


def kernel(q, k, v):
    assert q.shape == (B, H, N, D), q.shape
    if "nc" not in _CACHE:
        _CACHE["nc"] = build_nc()
    nc = _CACHE["nc"]

    full = pack_full_inputs(q, k, v)
    in_maps = []
    for i in range(8):
        in_maps.append({
            "qt": full["qt"][N_PAIR * i:N_PAIR * (i + 1)],
            "kt": full["kt"][N_PAIR * i:N_PAIR * (i + 1)],
            "vp": full["vp"][BH_PER_CORE * i:BH_PER_CORE * (i + 1)],
            "qr": full["qr"][NREM * i:NREM * (i + 1)],
            "kr": full["kr"][NREM * i:NREM * (i + 1)],
            "vr": full["vr"][NREM * i:NREM * (i + 1)],
        })

    # Retries: rapid repeated executions occasionally wedge a core with a
    # transient NRT_EXEC_UNIT_UNRECOVERABLE; a fresh attempt recovers, but
    # the device sometimes needs tens of seconds to settle — back off.
    import time
    res = None
    backoffs = [3.0, 10.0, 25.0, 45.0]
    for attempt in range(len(backoffs) + 1):
        try:
            res = run_bass_kernel_spmd(nc, in_maps, core_ids=list(range(8)))
            break
        except Exception:
            if attempt == len(backoffs):
                raise
            time.sleep(backoffs[attempt])

    om_full = np.concatenate([np.asarray(res.results[i]["om"]) for i in range(8)])
    orr_full = np.concatenate([np.asarray(res.results[i]["orr"]) for i in range(8)])
    return unpack_full_outputs(om_full, orr_full)
